# revision 16
# baseline (speedup 1.0000x reference)
"""Trainium2 Bass kernel for nn_FeatureRefinement.

Reference computation (bs=16, vl=1024, ql=64, d=1024):
    corr = einsum('bqd,bvd->bqv', Q, V); scores = softmax(corr, axis=1)
    corr_matrix = einsum('bqv,qd->bvd', scores, cor_w)     # cor_w constant over q
    sentence    = WeightedPool(Q)                           # (bs, d)
    sim         = cosine(V, sentence) + log(video_mask)     # (bs, vl)
    features    = concat([V, sim*sim_w, sentence_bcast, corr_matrix], -1)
    out         = relu(features @ mixer_w + mixer_b)

Algebraic restructuring (exact up to fp rounding):
  - softmax over q sums to 1  =>  corr_matrix[b,v,:] == cor_v_w*cor_q_w  (constant)
  - sim_features @ W2  == sim[b,v] * (sim_w.T @ W2)        (rank-1)
  - pooled_query @ W3  == sentence[b] @ W3                 (rank-1 per batch)
  so   out[b,v,:] = relu(V[b,v,:] @ W1 + sim[b,v]*w2v + bias[b,:])
  All O(n^2) terms are computed on the host in fp32; the device runs ONLY
  the O(n^3) part.

TRANSPOSED layout (trace-driven): the device computes out^T[b, d, v].
With d on partitions, the rank-2 addend decomposes per (d-tile n):
  - bias[b, n*128:+128] is a per-partition [P,1] column -> folded into the
    scalar-engine Relu (activation bias operand), costing zero DVE work;
  - sim[b,v]*w2v: ONE DVE scalar_tensor_tensor per group:
    (simb * w2v_col) + psum, with the PSUM tile as in1.
  The host transposes the output back (cheap numpy).  This removes the v3
  bottleneck where 16 upfront [128,1024] DVE addend instructions (~1.4us
  each) blocked the PSUM-freeing close ops, stalling the PE >3.4us and
  re-throttling the HAM clock gate to half width.

Hardware schedule notes (measured):
  - ~6.7us fixed NEFF preamble; first DMA data lands ~3.5us after issue.
  - HAM: PE at half width until 3.4us of gapless matmul activity; idle
    >3.4us re-throttles.  8 junk matmuls from ~7.5us flip it right as the
    first operands land (~10.5us).
  - Per-queue throughput ~150 GB/s, in-order completion.
  - W1 k-major [8,128,1024] (pure reshape) on scalar; V^T k-chunks of
    batch 0 on sync, batch 1 on gpsimd.  Chunk k of (w1, vtb0) arrives
    ~10.3+1.7k us; the 8-group wave (b0, n0-3, h0/h1) consumes 1.7us per
    chunk -- evenly paced.  Steady state is PE-bound: 256 matmuls x 213ns.
"""
import sys

sys.path.insert(0, "/opt/trn_rl_repo")

import numpy as np
from contextlib import ExitStack

import concourse.bass as bass
import concourse.tile as tile
from concourse import bacc, mybir
from concourse.bass_utils import run_bass_kernel_spmd


def _install_ntff_shim():
    """This container's antenv lacks axon_hooks; if tracing is requested
    (BASS_TRACE=1), run_bass_kernel_spmd would crash importing it. Provide
    the hook via trn_agent_boot's ctypes helper, and keep the trace
    post-processing local (no bucket uploads)."""
    import types
    try:
        import antenv  # noqa: F401
        import antenv.axon_hooks  # noqa: F401
        return  # already present
    except ImportError:
        pass
    try:
        import trn_agent_boot.trn_boot as _tb
        hook = _tb._ntff_profile_via_ctypes("/opt/axon/libaxon_pjrt.so")
        mod = types.ModuleType("antenv.axon_hooks")
        mod.get_axon_ntff_profile_hook = lambda: hook
        sys.modules["antenv.axon_hooks"] = mod
        from concourse import bass_utils as _bu
        _orig = _bu.upload_artifacts

        def _safe_upload(tmpdir):
            try:
                return _orig(tmpdir)
            except Exception:
                return f"file://{tmpdir}"

        _bu.upload_artifacts = _safe_upload
    except Exception:
        pass


_install_ntff_shim()

F32 = mybir.dt.float32
F16 = mybir.dt.float16
ALU = mybir.AluOpType
AF = mybir.ActivationFunctionType

BS, VL, QL, D = 16, 1024, 64, 1024
NCORES = 8
BPC = BS // NCORES          # batches per core
KC = D // 128               # contraction chunks
ND = D // 128               # output d-tiles (psum partition tiles)
NEG_INF = -1e30

N_WARM = 8                  # junk matmuls to warm the PE HAM gate
WAVE_N = 4                  # batch-0 d-tiles in the chunk-chasing wave


def _build_program():
    nc = bacc.Bacc("TRN2", target_bir_lowering=False, debug=False,
                   num_devices=NCORES)

    w1_d = nc.dram_tensor("w1", [KC, 128, D], F16, kind="ExternalInput").ap()
    vtb_d = nc.dram_tensor("vtb", [BPC, KC, 128, VL], F16,
                           kind="ExternalInput").ap()
    simb_d = nc.dram_tensor("simb", [BPC, 128, VL], F16,
                            kind="ExternalInput").ap()
    # cols[:, 0:8] = w2v columns; cols[:, 8+b*8+n] = bias[b] column n
    cols_d = nc.dram_tensor("cols", [128, ND + BPC * ND], F32,
                            kind="ExternalInput").ap()
    out_d = nc.dram_tensor("out", [BPC, D, VL], F16, kind="ExternalOutput").ap()

    with tile.TileContext(nc) as tc, ExitStack() as ctx:
        singles = ctx.enter_context(tc.tile_pool(name="singles", bufs=1))
        vtp = ctx.enter_context(tc.tile_pool(name="vtp", bufs=BPC * KC))
        opool = ctx.enter_context(tc.tile_pool(name="opool", bufs=4))
        tpool = ctx.enter_context(tc.tile_pool(name="tpool", bufs=4))
        psOut = ctx.enter_context(tc.tile_pool(name="psOut", bufs=8,
                                               space="PSUM"))

        w1_sb = singles.tile([128, KC, D], F16)
        vt = [[vtp.tile([128, VL], F16, tag="vt", name=f"vt{b}_{k}")
               for k in range(KC)] for b in range(BPC)]
        simb = singles.tile([128, BPC, VL], F16)
        cols = singles.tile([128, ND + BPC * ND], F32)

        # memset first on gpsimd: it exits the NEFF preamble ~0.7us before
        # the vector engine, so the junk stream (HAM flip) starts sooner
        warm16 = singles.tile([128, 512], F16)
        nc.gpsimd.memset(warm16, 0.0)

        # scalar: the 8 k-major W1 chunks, then half the batch-1 chunks.
        # sync mirrors with batch-0 chunks.  Keeping the batch-1 stream
        # BEHIND the wave-critical chunks (instead of on a third parallel
        # queue) preserves ~150 GB/s per queue during the wave; b1 chunks
        # still land (~31us) well before the b1 groups run (~42us).
        for k in range(KC):
            nc.scalar.dma_start(out=w1_sb[:, k, :], in_=w1_d[k])
        for k in range(KC):
            nc.sync.dma_start(out=vt[0][k], in_=vtb_d[0, k])
        for k in range(0, KC, 2):
            nc.scalar.dma_start(out=vt[1][k], in_=vtb_d[1, k])
        for k in range(1, KC, 2):
            nc.sync.dma_start(out=vt[1][k], in_=vtb_d[1, k])
        # gpsimd: the small close operands (needed at first close ~25us),
        # then it is free to carry every output store.  simb is split per
        # batch so the b1 half does not contend during the wave window.
        nc.gpsimd.dma_start(out=cols, in_=cols_d)
        nc.gpsimd.dma_start(out=simb[:, 0, :], in_=simb_d[0])
        nc.gpsimd.dma_start(out=simb[:, 1, :], in_=simb_d[1])

        # ================= PE HAM warmup ===========================
        for r in range(N_WARM):
            warm_ps = psOut.tile([128, 512], F32, tag="o_ps", name=f"warm{r}")
            nc.tensor.matmul(warm_ps, warm16[:, 0:128], warm16,
                             start=True, stop=True)

        # ================= matmul stream ===========================
        ps_of = {}
        out_sb = {}

        def open_group(b, n, h):
            ps_of[(b, n, h)] = psOut.tile([128, 512], F32, tag="o_ps",
                                          name=f"ps{b}_{n}_{h}")

        def mm(b, n, h, k):
            nc.tensor.matmul(ps_of[(b, n, h)],
                             w1_sb[:, k, n * 128:(n + 1) * 128],
                             vt[b][k][:, h * 512:(h + 1) * 512],
                             start=(k == 0), stop=(k == KC - 1))

        def close_group(b, n, h):
            ps = ps_of.pop((b, n, h))
            if (b, n) not in out_sb:
                out_sb[(b, n)] = opool.tile([128, VL], F16, tag="o16",
                                            name=f"o16_{b}_{n}")
            ot = out_sb[(b, n)]
            sl = slice(h * 512, (h + 1) * 512)
            tmp = tpool.tile([128, 512], F16, tag="tmp", name=f"tmp{b}{n}{h}")
            # tmp = sim[b,v]*w2v[n*128+p] + psum
            nc.vector.scalar_tensor_tensor(
                out=tmp, in0=simb[:, b, sl], scalar=cols[:, n:n + 1],
                in1=ps, op0=ALU.mult, op1=ALU.add)
            # out = max(tmp + bias[b, n*128+p], 0)  (per-partition bias)
            bc = ND + b * ND + n
            nc.vector.tensor_scalar(out=ot[:, sl], in0=tmp,
                                    scalar1=cols[:, bc:bc + 1], scalar2=0.0,
                                    op0=ALU.add, op1=ALU.max)
            if (b, n) == (1, ND - 1):
                # final tile: store each half as soon as it closes; the h1
                # half is further split across two queues (shorter tail)
                dst = out_d[b, n * 128:(n + 1) * 128, :]
                if h == 0:
                    nc.gpsimd.dma_start(out=dst[:, 0:512], in_=ot[:, 0:512])
                else:
                    nc.gpsimd.dma_start(out=dst[:, 512:768],
                                        in_=ot[:, 512:768])
                    nc.sync.dma_start(out=dst[:, 768:1024],
                                      in_=ot[:, 768:1024])
                    out_sb.pop((b, n))
            elif h == 1:
                emit_store(b, n)

        st_cnt = [0]

        def emit_store(b, n):
            ot = out_sb.pop((b, n))
            dst = out_d[b, n * 128:(n + 1) * 128, :]
            nc.gpsimd.dma_start(out=dst, in_=ot)
            st_cnt[0] += 1

        # wave: batch 0, d-tiles 0..WAVE_N-1, both halves; k-PAIR rounds
        # chase the parallel arrival of (w1[k], vtb0[k]) while keeping
        # same-bank runs of 2 (bank alternation costs ~46ns/matmul).
        for n in range(WAVE_N):
            for h in range(2):
                open_group(0, n, h)
        for kp in range(0, KC, 2):
            for n in range(WAVE_N):
                for h in range(2):
                    mm(0, n, h, kp)
                    mm(0, n, h, kp + 1)
        for n in range(WAVE_N):
            for h in range(2):
                close_group(0, n, h)
        # steady state: h-outer keeps 8 consecutive matmuls on one PSUM
        # bank (per-instruction bank alternation costs ~46ns/matmul in the
        # PE depth-cycling path).
        rest = [(0, n) for n in range(WAVE_N, ND)] + \
               [(1, n) for n in range(ND)]
        for b, n in rest:
            for h in range(2):
                open_group(b, n, h)
            for h in range(2):
                for k in range(KC):
                    mm(b, n, h, k)
            for h in range(2):
                close_group(b, n, h)

    nc.compile()
    return nc


_NC = None
_LAST_RESULTS = None


def _get_program():
    global _NC
    if _NC is None:
        _NC = _build_program()
    return _NC


def kernel(video_features, query_features, video_mask, query_mask,
           sim_w, cor_v_w, cor_q_w, pool_w, mixer_w, mixer_b):
    V = np.asarray(video_features, dtype=np.float32)
    Q = np.asarray(query_features, dtype=np.float32)
    vmask = np.asarray(video_mask, dtype=np.float32)
    qmask = np.asarray(query_mask, dtype=np.float32)
    sim_w = np.asarray(sim_w, dtype=np.float32)
    cor_v_w = np.asarray(cor_v_w, dtype=np.float32)
    cor_q_w = np.asarray(cor_q_w, dtype=np.float32)
    pool_w = np.asarray(pool_w, dtype=np.float32)
    mixer_w = np.asarray(mixer_w, dtype=np.float32)
    mixer_b = np.asarray(mixer_b, dtype=np.float32)

    W1 = mixer_w[0:D]
    W2 = mixer_w[D:2 * D]
    W3 = mixer_w[2 * D:3 * D]
    W4 = mixer_w[3 * D:4 * D]

    # ---- host-side O(n^2) math in fp32 (exact reference semantics) ----
    alpha = Q @ pool_w[:, 0] + (1.0 - qmask) * NEG_INF          # (bs, ql)
    alpha = alpha - alpha.max(axis=1, keepdims=True)
    ea = np.exp(alpha)
    alphas = ea / ea.sum(axis=1, keepdims=True)
    sentence = np.einsum('bqd,bq->bd', Q, alphas)               # (bs, d)
    dot = np.einsum('bvd,bd->bv', V, sentence)                  # (bs, vl)
    vn = np.maximum(np.linalg.norm(V, axis=-1), 1e-8)
    sn = np.maximum(np.linalg.norm(sentence, axis=-1), 1e-8)
    sim = dot / (vn * sn[:, None]) + np.log(vmask + 1e-45)      # (bs, vl)
    w2v = sim_w[:, 0] @ W2                                      # (d,)
    cor_vec = cor_v_w[0] * cor_q_w[0, 0]
    bias = sentence @ W3 + (cor_vec @ W4 + mixer_b)             # (bs, d)

    # ---- device layouts ----
    W1k = np.ascontiguousarray(W1.reshape(KC, 128, D)).astype(np.float16)
    v16 = V.astype(np.float16)
    sim16 = sim.astype(np.float16)
    w2vc = np.ascontiguousarray(w2v.reshape(ND, 128).T).astype(np.float32)

    nc = _get_program()
    in_maps = []
    for c in range(NCORES):
        sl = slice(c * BPC, (c + 1) * BPC)
        # vtb[b,k,p,v] = V[b, v, k*128+p]
        vtb = np.ascontiguousarray(
            v16[sl].transpose(0, 2, 1)).reshape(BPC, KC, 128, VL)
        simb = np.ascontiguousarray(
            np.broadcast_to(sim16[sl][:, None, :], (BPC, 128, VL)))
        cols = np.empty((128, ND + BPC * ND), dtype=np.float32)
        cols[:, 0:ND] = w2vc
        cols[:, ND:] = bias[sl].reshape(BPC * ND, 128).T
        in_maps.append({"w1": W1k, "vtb": vtb, "simb": simb, "cols": cols})
    res = run_bass_kernel_spmd(nc, in_maps, core_ids=list(range(NCORES)))
    global _LAST_RESULTS
    _LAST_RESULTS = res
    outT = np.concatenate([res.results[c]["out"] for c in range(NCORES)],
                          axis=0)                               # (bs, d, vl)
    return np.ascontiguousarray(outT.swapaxes(1, 2)).astype(np.float32)
